# revision 15
# baseline (speedup 1.0000x reference)
"""CrossAttention Trainium2 kernel (8 NeuronCores, SPMD), v2.

Problem: x [4,256,64,64], context [4,512,32,32], 8 heads x 64 dim,
q = Wq@x, k = Wk@ctx, v = Wv@ctx, attn = softmax(q^T k / 8), out = Wo@(v attn^T) + bo.

Sharding: fully data-parallel over (batch, query-spatial-half) -> 8 shards.
Zero collectives.  ScalarE exp (16.8M elem/core) is the roofline engine.

v2 structure vs v1:
  - 4 coalesced input DMAs (host-packed blobs) instead of 22.
  - PE warm-up matmuls during the input DMA so HAM reaches 8/8 early.
  - Minimal critical path to first exp: K(hp0) + Q(hp0,it0) only; V/K/Q
    projections interleaved into the first i-tile's attention blocks.
  - Per-(it,hp) normalization chains (den row -> SBUF-SBUF gather DMA ->
    lane-parallel reciprocal -> DRAM scatter -> partition-broadcast DMA),
    software-pipelined 4 blocks deep; f16 throughout.
  - Output projection for i-tile t emitted inside block (t+1, hp3).
  - Optional DVE "fast-exp" offload: exp(x) ~= bitcast_f16(i16(x*A + B)),
    computed by one tensor_scalar per offloaded slab to relieve ScalarE.
"""
import os
import sys
import numpy as np

for _p in ("/opt/trn_rl_repo", "/root/.axon_site/_ro/trn_rl_repo"):
    if os.path.isdir(_p) and _p not in sys.path:
        sys.path.insert(0, _p)

import concourse.bass as bass
import concourse.mybir as mybir
from concourse.tile import TileContext
from concourse.bass_utils import run_bass_kernel_spmd

F32 = mybir.dt.float32
F16 = mybir.dt.float16
I16 = mybir.dt.int16
EXP = mybir.ActivationFunctionType.Exp

B, H, D = 4, 8, 64
EQ, EK = 256, 512
NQ, NK = 2048, 1024
OC = 256
SCALE = D ** -0.5
IT, JT = NQ // 512, NK // 128

# DVE fast-exp offload: which jt slabs go to VectorE instead of ScalarE.
OFFLOAD = tuple(int(x) for x in os.environ.get("KOFFLOAD", "").split(",") if x != "")
FEXP_A = float(np.log2(np.e) * 1024.0 * SCALE)
FEXP_B = 15360.0 - 36.0


def _split_excess_waits(nc, max_waits=1):
    """This walrus build rejects instructions carrying >max_waits sem waits;
    move the extras onto standalone nops just before (same engine, in-order,
    so semantics are unchanged)."""
    n_new = 0
    for f in nc.m.functions:
        for bb in f.blocks:
            insts = list(bb.instructions)
            out = []
            changed = False
            for inst in insts:
                si = inst.sync_info
                if si is not None and si.on_wait and len(si.on_wait) > max_waits:
                    waits = list(si.on_wait)
                    for w in waits[:-max_waits]:
                        nop = mybir.InstNoOp(
                            name=f"I-splitw-{n_new}",
                            sync_info=mybir.SyncInfo(on_wait=[w], on_update=[]),
                        )
                        nop.engine = inst.engine
                        n_new += 1
                        out.append(nop)
                        nc.register_instruction(nop, overwrite=True)
                    si.on_wait = waits[-max_waits:]
                    inst.sync_info = si
                    changed = True
                out.append(inst)
            if changed:
                bb.instructions.clear()
                bb.instructions.extend(out)
    return n_new


def _build():
    nc = bass.Bass()
    # host-packed blobs (f16):
    #  blob1 [128, 6144]: ctx (4 ec-chunks of 1024) + WkT (4 ec-chunks of 512)
    #  blob2 [128, 5120]: x   (2 ec-chunks of 2048) + WqT (2 ec-chunks of 512)
    #  blob3 [128, 3072]: WvT (4 ec-chunks of 512)  + WoT (4 cc-chunks of 256)
    blobA = nc.declare_dram_parameter("blobA", [128, 4608], F16, isOutput=False)
    blobA2 = nc.declare_dram_parameter("blobA2", [128, 1280], F16, isOutput=False)
    blobB1 = nc.declare_dram_parameter("blobB1", [128, 4352], F16, isOutput=False)
    blobB2 = nc.declare_dram_parameter("blobB2", [128, 4096], F16, isOutput=False)
    bo2 = nc.declare_dram_parameter("bo2", [128, 2], F32, isOutput=False)
    y = nc.declare_dram_parameter("y", [OC, NQ], F32, isOutput=True)

    sscratch2 = nc.dram_tensor("sscratch2", [IT * 4 * 1024], F16)

    with TileContext(nc) as tc:
        with (
            tc.tile_pool(name="consts", bufs=1) as cp,
            tc.tile_pool(name="qkv", bufs=1) as qp,
            tc.tile_pool(name="exps", bufs=5) as ep,
            tc.tile_pool(name="avrp", bufs=8) as avrp,
            tc.tile_pool(name="bcastp", bufs=6) as bcp,
            tc.tile_pool(name="avnp", bufs=6) as avnp,
            tc.tile_pool(name="work", bufs=4) as wp,
            tc.tile_pool(name="slab", bufs=2, space="PSUM") as slabp,
            tc.tile_pool(name="avp", bufs=1, space="PSUM") as avp,
            tc.tile_pool(name="yp", bufs=2, space="PSUM") as yp,
        ):
            bigA = cp.tile([128, 4608], F16, tag="bigA")
            bigA2 = cp.tile([128, 1280], F16, tag="bigA2")
            bigB1 = cp.tile([128, 4352], F16, tag="bigB1")
            bigB2 = cp.tile([128, 4096], F16, tag="bigB2")
            bo_col = cp.tile([128, 2], F32, tag="bo_col")
            dummy = cp.tile([128, 512], F16, tag="dummy")
            ones_f32 = cp.tile([128, JT * H], F32, tag="ones_f32")
            ones16 = cp.tile([1, 64], F16, tag="ones16")

            # PE warm-up: runs during the input DMAs (no data deps), flips
            # HAM to 8/8 before the first real matmul.
            nc.vector.memset(dummy, 0.0)
            nc.vector.memset(ones_f32, 1.0)
            nc.vector.memset(ones16, 1.0)
            warm = yp.tile([128, 512], F32, tag="yps")
            for i in range(12):
                nc.tensor.matmul(warm, lhsT=dummy[:, 0:128], rhs=dummy,
                                 start=(i == 0), stop=(i == 11))

            nc.sync.dma_start(out=bigA, in_=blobA[:, :])
            nc.sync.dma_start(out=bigA2, in_=blobA2[:, :])
            nc.sync.dma_start(out=bigB1, in_=blobB1[:, :])
            nc.sync.dma_start(out=bigB2, in_=blobB2[:, :])
            nc.sync.dma_start(out=bo_col, in_=bo2[:, :])

            def ctx_sb(ec, lo, hi):
                return bigA[:, ec * 1024 + lo: ec * 1024 + hi]

            def wkt(ec, hp):
                if hp == 0:
                    return bigA[:, 4096 + ec * 128: 4096 + (ec + 1) * 128]
                o = 2048 + ec * 384 + (hp - 1) * 128
                return bigB1[:, o: o + 128]

            def wqt(ec, hp):
                if hp == 0:
                    return bigA2[:, ec * 128: (ec + 1) * 128]
                o = 3584 + ec * 384 + (hp - 1) * 128
                return bigB1[:, o: o + 128]

            def x_sb(ec, it, lo, hi):
                if it == 0:
                    return bigA2[:, 256 + ec * 512 + lo: 256 + ec * 512 + hi]
                o = ec * 1536 + (it - 1) * 512
                return bigB2[:, o + lo: o + hi]

            def wvt(ec):
                return bigB1[:, ec * 512:(ec + 1) * 512]

            def wot(cc, lo, hi):
                return bigB2[:, 3072 + cc * 256 + lo: 3072 + cc * 256 + hi]

            # persistent activations
            q_sb = qp.tile([128, 4 * NQ], F16, tag="q_sb")      # [hp, i]
            k_sb = qp.tile([128, 4 * NK], F16, tag="k_sb")      # [hp, j]
            vt_sb = qp.tile([128, JT * 520], F16, tag="vt_sb")  # [jt, h*65 + c]

            # ones columns of vt (col 64 of each 65-block) -> softmax denom
            vt_4d = vt_sb.rearrange("p (j h c) -> p j h c", j=JT, h=H)
            nc.vector.tensor_copy(
                vt_4d[:, :, :, 64:65],
                ones_f32.rearrange("p (j h) -> p j h", j=JT).unsqueeze(-1))

            def k_proj(hp):
                for ntile in range(NK // 512):
                    pk = yp.tile([128, 512], F32, tag="yps")
                    for ec in range(4):
                        nc.tensor.matmul(
                            pk,
                            lhsT=wkt(ec, hp),
                            rhs=ctx_sb(ec, ntile * 512, (ntile + 1) * 512),
                            start=(ec == 0), stop=(ec == 3))
                    nc.vector.tensor_copy(
                        k_sb[:, hp * NK + ntile * 512: hp * NK + (ntile + 1) * 512], pk)

            def q_proj_tile(hp, ntile):
                pq = yp.tile([128, 512], F32, tag="yps")
                for ec in range(2):
                    nc.tensor.matmul(
                        pq,
                        lhsT=wqt(ec, hp),
                        rhs=x_sb(ec, ntile, 0, 512),
                        start=(ec == 0), stop=(ec == 1))
                nc.vector.tensor_copy(
                    q_sb[:, hp * NQ + ntile * 512: hp * NQ + (ntile + 1) * 512], pq)

            def v_proj(jt):
                pv = yp.tile([128, 512], F32, tag="yps")
                for ec in range(4):
                    nc.tensor.matmul(
                        pv,
                        lhsT=ctx_sb(ec, jt * 128, (jt + 1) * 128),
                        rhs=wvt(ec),
                        start=(ec == 0), stop=(ec == 3))
                vt_t = vt_sb[:, jt * 520:(jt + 1) * 520].rearrange(
                    "p (h c) -> p h c", h=H)[:, :, 0:64]
                nc.vector.tensor_copy(vt_t, pv.rearrange("p (h c) -> p h c", c=64))

            k_proj(0)
            q_proj_tile(0, 0)

            def sim_emit(hp, it, jt):
                slab = slabp.tile([128, 1024], F32, tag="slab")
                ks = slice(hp * NK + jt * 128, hp * NK + (jt + 1) * 128)
                qs = slice(hp * NQ + it * 512, hp * NQ + (it + 1) * 512)
                nc.tensor.matmul(
                    slab[:, 0:512], lhsT=k_sb[0:64, ks], rhs=q_sb[0:64, qs],
                    start=True, stop=True, tile_position=(0, 0))
                nc.tensor.matmul(
                    slab[:, 512:1024], lhsT=k_sb[64:128, ks], rhs=q_sb[64:128, qs],
                    start=True, stop=True, tile_position=(64, 0))
                return slab

            avr_tiles = {}    # (it, hp) -> avr [65, 1024] f16
            bcast_tiles = {}  # (it, hp) -> bcast [64, 1024] f16
            avn_tiles = {}    # (it, hp) -> avn [128, 512] f16

            def norm_mul(it, hp):
                avr = avr_tiles.pop((it, hp))
                bcast = bcast_tiles.pop((it, hp))
                avn = avnp.tile([128, 512], F16, tag="avn")
                if isinstance(bcast, list):
                    nc.vector.tensor_mul(avn[0:64, :], avr[0:64, 0:512], bcast[0])
                    nc.vector.tensor_mul(avn[64:128, :], avr[0:64, 512:1024], bcast[1])
                else:
                    nc.vector.tensor_mul(avn[0:64, :], avr[0:64, 0:512], bcast[:, 0:512])
                    nc.vector.tensor_mul(avn[64:128, :], avr[0:64, 512:1024], bcast[:, 512:1024])
                avn_tiles[(it, hp)] = avn

            oproj_yps = {}

            def oproj_step(it, cc):
                # accumulate avn(it, cc) into the two output-channel psums
                avn = avn_tiles.pop((it, cc))
                for ob in range(2):
                    if cc == 0:
                        yps_t = yp.tile([128, 512], F32, tag="yps")
                        oproj_yps[(it, ob)] = yps_t
                    nc.tensor.matmul(
                        oproj_yps[(it, ob)],
                        lhsT=wot(cc, ob * 128, (ob + 1) * 128),
                        rhs=avn,
                        start=(cc == 0), stop=(cc == 3))
                if cc == 3:
                    for ob in range(2):
                        yps = oproj_yps.pop((it, ob))
                        ysb = wp.tile([128, 512], F32, tag="ysb")
                        nc.vector.tensor_scalar_add(ysb, yps, bo_col[:, ob:ob + 1])
                        nc.sync.dma_start(
                            out=y[ob * 128:(ob + 1) * 128, it * 512:(it + 1) * 512],
                            in_=ysb)

            def oproj(it):
                for cc in range(4):
                    oproj_step(it, cc)

            # ---- attention blocks ----
            for it in range(IT):
                for hp in range(4):
                    first_block = (it == 0 and hp == 0)
                    if it > 0 and not (it == 3 and hp == 3):
                        norm_mul(it - 1, hp)
                        if hp == 3 and it < 3:
                            oproj(it - 1)
                    if it == 3 and hp == 2:
                        # finish it=2 early so the yp pool frees up, then
                        # start it=3's output projection as chains complete
                        norm_mul(2, 3)
                        oproj(2)
                        norm_mul(3, 0)
                        oproj_step(3, 0)
                    if it == 3 and hp == 3:
                        norm_mul(3, 1)
                        oproj_step(3, 1)

                    av0 = avp.tile([65, 512], F32, tag="av0")
                    av1 = avp.tile([65, 512], F32, tag="av1")
                    slab = sim_emit(hp, it, 0)
                    for jt in range(JT):
                        nslab = sim_emit(hp, it, jt + 1) if jt + 1 < JT else None
                        if jt in OFFLOAD:
                            fx = ep.tile([128, 1024], I16, tag="fexp")
                            nc.vector.tensor_scalar(
                                fx, slab, FEXP_A, FEXP_B,
                                mybir.AluOpType.mult, mybir.AluOpType.add)
                            exps = fx.bitcast(F16)
                        else:
                            exps = ep.tile([128, 1024], F16, tag="exps")
                            nc.scalar.activation(exps, slab, EXP, bias=0.0, scale=SCALE)
                        if first_block:
                            v_proj(jt)
                        nc.tensor.matmul(
                            av0,
                            lhsT=vt_sb[:, jt * 520 + (2 * hp) * 65: jt * 520 + (2 * hp) * 65 + 65],
                            rhs=exps[:, 0:512],
                            start=(jt == 0), stop=(jt == JT - 1))
                        nc.tensor.matmul(
                            av1,
                            lhsT=vt_sb[:, jt * 520 + (2 * hp + 1) * 65: jt * 520 + (2 * hp + 1) * 65 + 65],
                            rhs=exps[:, 512:1024],
                            start=(jt == 0), stop=(jt == JT - 1))
                        slab = nslab

                    # drain AV banks (f16) + per-(it,hp) normalization chain
                    last = (it == 3 and hp == 3)
                    avr = avrp.tile([96 if last else 65, 1024], F16, tag="avr")
                    nc.vector.tensor_copy(avr[0:65, 0:512], av0)
                    nc.vector.tensor_copy(avr[0:65, 512:1024], av1)
                    avr_tiles[(it, hp)] = avr
                    if last:
                        # on-chip recip path: no DRAM bounce latency on the tail
                        ttile = wp.tile([32, 1024], F16, tag="ttile")
                        nc.vector.transpose(ttile, avr[64:96, :])
                        with nc.allow_low_precision(reason="f16 softmax denominators"):
                            nc.vector.reciprocal(
                                ttile.rearrange("p (b s) -> p b s", s=32)[:, :, 0:1],
                                ttile.rearrange("p (b s) -> p b s", s=32)[:, :, 0:1])
                        rrow = wp.tile([32, 1024], F16, tag="rrow")
                        nc.vector.transpose(rrow, ttile)
                        bc_pss = []
                        for half in range(2):
                            bc_ps = avp.tile([64, 512], F32, tag="av0" if half == 0 else "av1")
                            nc.tensor.matmul(
                                bc_ps, lhsT=ones16,
                                rhs=rrow[0:1, half * 512:(half + 1) * 512],
                                start=True, stop=True)
                            bc_pss.append(bc_ps)
                        bcast_tiles[(it, hp)] = bc_pss
                    else:
                        stile = wp.tile([128, 8], F16, tag="stile")
                        nc.sync.dma_start(out=stile, in_=avr[64:65, 0:1024])
                        stile_r = wp.tile([128, 8], F16, tag="stile_r")
                        with nc.allow_low_precision(reason="f16 softmax denominators"):
                            nc.vector.reciprocal(stile_r, stile)
                        base = (it * 4 + hp) * 1024
                        nc.sync.dma_start(
                            out=bass.AP(tensor=sscratch2, offset=base, ap=[[1, 1024]]),
                            in_=stile_r)
                        bcast = bcp.tile([64, 1024], F16, tag="bcast")
                        nc.sync.dma_start(
                            out=bcast,
                            in_=bass.AP(tensor=sscratch2, offset=base,
                                        ap=[[0, 64], [1, 1024]]))
                        bcast_tiles[(it, hp)] = bcast

                    if it < 3:
                        q_proj_tile(hp, it + 1)
                    if it == 0 and hp < 3:
                        k_proj(hp + 1)
                        q_proj_tile(hp + 1, 0)

            norm_mul(3, 2)
            oproj_step(3, 2)
            norm_mul(3, 3)
            oproj_step(3, 3)

    _split_excess_waits(nc)
    return nc


_CACHED = None


def kernel(x, context, Wq, Wk, Wv, Wo, bo):
    global _CACHED
    if _CACHED is None:
        _CACHED = _build()
    nc = _CACHED

    x = np.asarray(x, dtype=np.float32)
    context = np.asarray(context, dtype=np.float32)
    xf = x.reshape(B, EQ, 64 * 64)
    cf = context.reshape(B, EK, 32 * 32)
    WqT = np.asarray(Wq, np.float32).T.astype(np.float16)   # [EQ, 512]
    WkT = np.asarray(Wk, np.float32).T.astype(np.float16)   # [EK, 512]
    WvT = np.asarray(Wv, np.float32).T.astype(np.float16)   # [EK, 512]
    WoT = np.asarray(Wo, np.float32).T.astype(np.float16)   # [512, OC]
    bo = np.asarray(bo, np.float32)

    # blob1 per batch: ctx chunks + WkT chunks
    def chunks(a, n):
        # [n*128, F] -> [128, n*F] with chunk-major free layout
        return a.reshape(n, 128, -1).transpose(1, 0, 2).reshape(128, -1)

    wk4 = WkT.reshape(4, 128, 4, 128)     # [ec, p, hp, 128]
    wq2 = WqT.reshape(2, 128, 4, 128)     # [ec, p, hp, 128]
    wkt_hp0 = wk4[:, :, 0].transpose(1, 0, 2).reshape(128, 512)
    wqt_hp0 = wq2[:, :, 0].transpose(1, 0, 2).reshape(128, 256)
    wkt_r = wk4[:, :, 1:].transpose(1, 0, 2, 3).reshape(128, 1536)
    wqt_r = wq2[:, :, 1:].transpose(1, 0, 2, 3).reshape(128, 768)
    blobB1 = np.ascontiguousarray(
        np.concatenate([chunks(WvT, 4), wkt_r, wqt_r], axis=1))
    bo2 = np.ascontiguousarray(bo.reshape(2, 128).T)

    in_maps = []
    for core in range(8):
        b, half = core // 2, core % 2
        xh = xf[b, :, half * NQ:(half + 1) * NQ].astype(np.float16).reshape(2, 128, 4, 512)
        x_it0 = xh[:, :, 0].transpose(1, 0, 2).reshape(128, 1024)
        x_r = xh[:, :, 1:].transpose(1, 0, 2, 3).reshape(128, 3072)
        blobA = np.ascontiguousarray(np.concatenate(
            [chunks(cf[b].astype(np.float16), 4), wkt_hp0], axis=1))
        blobA2 = np.ascontiguousarray(np.concatenate([wqt_hp0, x_it0], axis=1))
        blobB2 = np.ascontiguousarray(
            np.concatenate([x_r, chunks(WoT, 4)], axis=1))
        in_maps.append({
            "blobA": blobA, "blobA2": blobA2, "blobB1": blobB1, "blobB2": blobB2,
            "bo2": bo2,
        })

    res = run_bass_kernel_spmd(nc, in_maps, list(range(8)))
    kernel.last_results = res

    out = np.empty((B, OC, 64 * 64), dtype=np.float32)
    for core in range(8):
        b, half = core // 2, core % 2
        out[b, :, half * NQ:(half + 1) * NQ] = res.results[core]["y"]
    return out.reshape(B, OC, 64, 64)


# revision 16
# speedup vs baseline: 1.0204x; 1.0204x over previous
"""CrossAttention Trainium2 kernel (8 NeuronCores, SPMD), v2.

Problem: x [4,256,64,64], context [4,512,32,32], 8 heads x 64 dim,
q = Wq@x, k = Wk@ctx, v = Wv@ctx, attn = softmax(q^T k / 8), out = Wo@(v attn^T) + bo.

Sharding: fully data-parallel over (batch, query-spatial-half) -> 8 shards.
Zero collectives.  ScalarE exp (16.8M elem/core) is the roofline engine.

v2 structure vs v1:
  - 4 coalesced input DMAs (host-packed blobs) instead of 22.
  - PE warm-up matmuls during the input DMA so HAM reaches 8/8 early.
  - Minimal critical path to first exp: K(hp0) + Q(hp0,it0) only; V/K/Q
    projections interleaved into the first i-tile's attention blocks.
  - Per-(it,hp) normalization chains (den row -> SBUF-SBUF gather DMA ->
    lane-parallel reciprocal -> DRAM scatter -> partition-broadcast DMA),
    software-pipelined 4 blocks deep; f16 throughout.
  - Output projection for i-tile t emitted inside block (t+1, hp3).
  - Optional DVE "fast-exp" offload: exp(x) ~= bitcast_f16(i16(x*A + B)),
    computed by one tensor_scalar per offloaded slab to relieve ScalarE.
"""
import os
import sys
import numpy as np

for _p in ("/opt/trn_rl_repo", "/root/.axon_site/_ro/trn_rl_repo"):
    if os.path.isdir(_p) and _p not in sys.path:
        sys.path.insert(0, _p)

import concourse.bass as bass
import concourse.mybir as mybir
from concourse.tile import TileContext
from concourse.bass_utils import run_bass_kernel_spmd

F32 = mybir.dt.float32
F16 = mybir.dt.float16
I16 = mybir.dt.int16
EXP = mybir.ActivationFunctionType.Exp

B, H, D = 4, 8, 64
EQ, EK = 256, 512
NQ, NK = 2048, 1024
OC = 256
SCALE = D ** -0.5
IT, JT = NQ // 512, NK // 128

# DVE fast-exp offload: which jt slabs go to VectorE instead of ScalarE.
OFFLOAD = tuple(int(x) for x in os.environ.get("KOFFLOAD", "").split(",") if x != "")
FEXP_A = float(np.log2(np.e) * 1024.0 * SCALE)
FEXP_B = 15360.0 - 36.0


def _split_excess_waits(nc, max_waits=1):
    """This walrus build rejects instructions carrying >max_waits sem waits;
    move the extras onto standalone nops just before (same engine, in-order,
    so semantics are unchanged)."""
    n_new = 0
    for f in nc.m.functions:
        for bb in f.blocks:
            insts = list(bb.instructions)
            out = []
            changed = False
            for inst in insts:
                si = inst.sync_info
                if si is not None and si.on_wait and len(si.on_wait) > max_waits:
                    waits = list(si.on_wait)
                    for w in waits[:-max_waits]:
                        nop = mybir.InstNoOp(
                            name=f"I-splitw-{n_new}",
                            sync_info=mybir.SyncInfo(on_wait=[w], on_update=[]),
                        )
                        nop.engine = inst.engine
                        n_new += 1
                        out.append(nop)
                        nc.register_instruction(nop, overwrite=True)
                    si.on_wait = waits[-max_waits:]
                    inst.sync_info = si
                    changed = True
                out.append(inst)
            if changed:
                bb.instructions.clear()
                bb.instructions.extend(out)
    return n_new


def _build():
    nc = bass.Bass()
    # host-packed blobs (f16):
    #  blob1 [128, 6144]: ctx (4 ec-chunks of 1024) + WkT (4 ec-chunks of 512)
    #  blob2 [128, 5120]: x   (2 ec-chunks of 2048) + WqT (2 ec-chunks of 512)
    #  blob3 [128, 3072]: WvT (4 ec-chunks of 512)  + WoT (4 cc-chunks of 256)
    blobA = nc.declare_dram_parameter("blobA", [128, 4608], F16, isOutput=False)
    blobA2 = nc.declare_dram_parameter("blobA2", [128, 1280], F16, isOutput=False)
    blobB1 = nc.declare_dram_parameter("blobB1", [128, 4352], F16, isOutput=False)
    blobB2 = nc.declare_dram_parameter("blobB2", [128, 4096], F16, isOutput=False)
    bo2 = nc.declare_dram_parameter("bo2", [128, 2], F32, isOutput=False)
    y = nc.declare_dram_parameter("y", [OC, NQ], F32, isOutput=True)

    sscratch2 = nc.dram_tensor("sscratch2", [IT * 4 * 1024], F16)

    with TileContext(nc) as tc:
        with (
            tc.tile_pool(name="consts", bufs=1) as cp,
            tc.tile_pool(name="qkv", bufs=1) as qp,
            tc.tile_pool(name="exps", bufs=4) as ep,
            tc.tile_pool(name="avrp", bufs=8) as avrp,
            tc.tile_pool(name="bcastp", bufs=6) as bcp,
            tc.tile_pool(name="avnp", bufs=6) as avnp,
            tc.tile_pool(name="work", bufs=4) as wp,
            tc.tile_pool(name="slab", bufs=2, space="PSUM") as slabp,
            tc.tile_pool(name="avp", bufs=1, space="PSUM") as avp,
            tc.tile_pool(name="yp", bufs=2, space="PSUM") as yp,
        ):
            bigA = cp.tile([128, 4608], F16, tag="bigA")
            bigA2 = cp.tile([128, 1280], F16, tag="bigA2")
            bigB1 = cp.tile([128, 4352], F16, tag="bigB1")
            bigB2 = cp.tile([128, 4096], F16, tag="bigB2")
            bo_col = cp.tile([128, 2], F32, tag="bo_col")
            dummy = cp.tile([128, 512], F16, tag="dummy")
            ones_f32 = cp.tile([128, JT * H], F32, tag="ones_f32")
            ones16 = cp.tile([1, 64], F16, tag="ones16")

            # PE warm-up: runs during the input DMAs (no data deps), flips
            # HAM to 8/8 before the first real matmul.
            nc.vector.memset(dummy, 0.0)
            nc.vector.memset(ones_f32, 1.0)
            nc.vector.memset(ones16, 1.0)
            warm = yp.tile([128, 512], F32, tag="yps")
            for i in range(18):
                nc.tensor.matmul(warm, lhsT=dummy[:, 0:128], rhs=dummy,
                                 start=(i == 0), stop=(i == 17))

            nc.sync.dma_start(out=bigA, in_=blobA[:, :])
            nc.sync.dma_start(out=bigA2, in_=blobA2[:, :])
            nc.sync.dma_start(out=bigB1, in_=blobB1[:, :])
            nc.sync.dma_start(out=bigB2, in_=blobB2[:, :])
            nc.sync.dma_start(out=bo_col, in_=bo2[:, :])

            def ctx_sb(ec, lo, hi):
                return bigA[:, ec * 1024 + lo: ec * 1024 + hi]

            def wkt(ec, hp):
                if hp == 0:
                    return bigA[:, 4096 + ec * 128: 4096 + (ec + 1) * 128]
                o = 2048 + ec * 384 + (hp - 1) * 128
                return bigB1[:, o: o + 128]

            def wqt(ec, hp):
                if hp == 0:
                    return bigA2[:, ec * 128: (ec + 1) * 128]
                o = 3584 + ec * 384 + (hp - 1) * 128
                return bigB1[:, o: o + 128]

            def x_sb(ec, it, lo, hi):
                if it == 0:
                    return bigA2[:, 256 + ec * 512 + lo: 256 + ec * 512 + hi]
                o = ec * 1536 + (it - 1) * 512
                return bigB2[:, o + lo: o + hi]

            def wvt(ec):
                return bigB1[:, ec * 512:(ec + 1) * 512]

            def wot(cc, lo, hi):
                return bigB2[:, 3072 + cc * 256 + lo: 3072 + cc * 256 + hi]

            # persistent activations
            q_sb = qp.tile([128, 4 * NQ], F16, tag="q_sb")      # [hp, i]
            k_sb = qp.tile([128, 4 * NK], F16, tag="k_sb")      # [hp, j]
            vt_sb = qp.tile([128, JT * 520], F16, tag="vt_sb")  # [jt, h*65 + c]

            # ones columns of vt (col 64 of each 65-block) -> softmax denom
            vt_4d = vt_sb.rearrange("p (j h c) -> p j h c", j=JT, h=H)
            nc.vector.tensor_copy(
                vt_4d[:, :, :, 64:65],
                ones_f32.rearrange("p (j h) -> p j h", j=JT).unsqueeze(-1))

            def k_proj(hp):
                for ntile in range(NK // 512):
                    pk = yp.tile([128, 512], F32, tag="yps")
                    for ec in range(4):
                        nc.tensor.matmul(
                            pk,
                            lhsT=wkt(ec, hp),
                            rhs=ctx_sb(ec, ntile * 512, (ntile + 1) * 512),
                            start=(ec == 0), stop=(ec == 3))
                    nc.vector.tensor_copy(
                        k_sb[:, hp * NK + ntile * 512: hp * NK + (ntile + 1) * 512], pk)

            def q_proj_tile(hp, ntile):
                pq = yp.tile([128, 512], F32, tag="yps")
                for ec in range(2):
                    nc.tensor.matmul(
                        pq,
                        lhsT=wqt(ec, hp),
                        rhs=x_sb(ec, ntile, 0, 512),
                        start=(ec == 0), stop=(ec == 1))
                nc.vector.tensor_copy(
                    q_sb[:, hp * NQ + ntile * 512: hp * NQ + (ntile + 1) * 512], pq)

            def v_proj(jt):
                pv = yp.tile([128, 512], F32, tag="yps")
                for ec in range(4):
                    nc.tensor.matmul(
                        pv,
                        lhsT=ctx_sb(ec, jt * 128, (jt + 1) * 128),
                        rhs=wvt(ec),
                        start=(ec == 0), stop=(ec == 3))
                vt_t = vt_sb[:, jt * 520:(jt + 1) * 520].rearrange(
                    "p (h c) -> p h c", h=H)[:, :, 0:64]
                nc.vector.tensor_copy(vt_t, pv.rearrange("p (h c) -> p h c", c=64))

            k_proj(0)
            q_proj_tile(0, 0)

            def sim_emit(hp, it, jt):
                slab = slabp.tile([128, 1024], F32, tag="slab")
                ks = slice(hp * NK + jt * 128, hp * NK + (jt + 1) * 128)
                qs = slice(hp * NQ + it * 512, hp * NQ + (it + 1) * 512)
                nc.tensor.matmul(
                    slab[:, 0:512], lhsT=k_sb[0:64, ks], rhs=q_sb[0:64, qs],
                    start=True, stop=True, tile_position=(0, 0))
                nc.tensor.matmul(
                    slab[:, 512:1024], lhsT=k_sb[64:128, ks], rhs=q_sb[64:128, qs],
                    start=True, stop=True, tile_position=(64, 0))
                return slab

            avr_tiles = {}    # (it, hp) -> avr [65, 1024] f16
            bcast_tiles = {}  # (it, hp) -> bcast [64, 1024] f16
            avn_tiles = {}    # (it, hp) -> avn [128, 512] f16

            def norm_mul(it, hp):
                avr = avr_tiles.pop((it, hp))
                bcast = bcast_tiles.pop((it, hp))
                avn = avnp.tile([128, 512], F16, tag="avn")
                if isinstance(bcast, list):
                    nc.vector.tensor_mul(avn[0:64, :], avr[0:64, 0:512], bcast[0])
                    nc.vector.tensor_mul(avn[64:128, :], avr[0:64, 512:1024], bcast[1])
                else:
                    nc.vector.tensor_mul(avn[0:64, :], avr[0:64, 0:512], bcast[:, 0:512])
                    nc.vector.tensor_mul(avn[64:128, :], avr[0:64, 512:1024], bcast[:, 512:1024])
                avn_tiles[(it, hp)] = avn

            oproj_yps = {}

            def oproj_step(it, cc):
                # accumulate avn(it, cc) into the two output-channel psums
                avn = avn_tiles.pop((it, cc))
                for ob in range(2):
                    if cc == 0:
                        yps_t = yp.tile([128, 512], F32, tag="yps")
                        oproj_yps[(it, ob)] = yps_t
                    nc.tensor.matmul(
                        oproj_yps[(it, ob)],
                        lhsT=wot(cc, ob * 128, (ob + 1) * 128),
                        rhs=avn,
                        start=(cc == 0), stop=(cc == 3))
                if cc == 3:
                    for ob in range(2):
                        yps = oproj_yps.pop((it, ob))
                        ysb = wp.tile([128, 512], F32, tag="ysb")
                        nc.vector.tensor_scalar_add(ysb, yps, bo_col[:, ob:ob + 1])
                        nc.sync.dma_start(
                            out=y[ob * 128:(ob + 1) * 128, it * 512:(it + 1) * 512],
                            in_=ysb)

            def oproj(it):
                for cc in range(4):
                    oproj_step(it, cc)

            # ---- attention blocks ----
            for it in range(IT):
                for hp in range(4):
                    first_block = (it == 0 and hp == 0)
                    if it > 0 and not (it == 3 and hp == 3):
                        norm_mul(it - 1, hp)
                        if hp == 3 and it < 3:
                            oproj(it - 1)
                    if it == 3 and hp == 2:
                        # finish it=2 early so the yp pool frees up, then
                        # start it=3's output projection as chains complete
                        norm_mul(2, 3)
                        oproj(2)
                        norm_mul(3, 0)
                        oproj_step(3, 0)
                    if it == 3 and hp == 3:
                        norm_mul(3, 1)
                        oproj_step(3, 1)

                    av0 = avp.tile([65, 512], F32, tag="av0")
                    av1 = avp.tile([65, 512], F32, tag="av1")
                    slab = sim_emit(hp, it, 0)
                    for jt in range(JT):
                        nslab = sim_emit(hp, it, jt + 1) if jt + 1 < JT else None
                        if jt in OFFLOAD:
                            fx = ep.tile([128, 1024], I16, tag="fexp")
                            nc.vector.tensor_scalar(
                                fx, slab, FEXP_A, FEXP_B,
                                mybir.AluOpType.mult, mybir.AluOpType.add)
                            exps = fx.bitcast(F16)
                        else:
                            exps = ep.tile([128, 1024], F16, tag="exps")
                            nc.scalar.activation(exps, slab, EXP, bias=0.0, scale=SCALE)
                        if first_block:
                            v_proj(jt)
                        nc.tensor.matmul(
                            av0,
                            lhsT=vt_sb[:, jt * 520 + (2 * hp) * 65: jt * 520 + (2 * hp) * 65 + 65],
                            rhs=exps[:, 0:512],
                            start=(jt == 0), stop=(jt == JT - 1))
                        nc.tensor.matmul(
                            av1,
                            lhsT=vt_sb[:, jt * 520 + (2 * hp + 1) * 65: jt * 520 + (2 * hp + 1) * 65 + 65],
                            rhs=exps[:, 512:1024],
                            start=(jt == 0), stop=(jt == JT - 1))
                        slab = nslab

                    # drain AV banks (f16) + per-(it,hp) normalization chain
                    last = (it == 3 and hp == 3)
                    avr = avrp.tile([96 if last else 65, 1024], F16, tag="avr")
                    nc.vector.tensor_copy(avr[0:65, 0:512], av0)
                    nc.vector.tensor_copy(avr[0:65, 512:1024], av1)
                    avr_tiles[(it, hp)] = avr
                    if last:
                        # on-chip recip path: no DRAM bounce latency on the tail
                        ttile = wp.tile([32, 1024], F16, tag="ttile")
                        nc.vector.transpose(ttile, avr[64:96, :])
                        with nc.allow_low_precision(reason="f16 softmax denominators"):
                            nc.vector.reciprocal(
                                ttile.rearrange("p (b s) -> p b s", s=32)[:, :, 0:1],
                                ttile.rearrange("p (b s) -> p b s", s=32)[:, :, 0:1])
                        rrow = wp.tile([32, 1024], F16, tag="rrow")
                        nc.vector.transpose(rrow, ttile)
                        bc_pss = []
                        for half in range(2):
                            bc_ps = avp.tile([64, 512], F32, tag="av0" if half == 0 else "av1")
                            nc.tensor.matmul(
                                bc_ps, lhsT=ones16,
                                rhs=rrow[0:1, half * 512:(half + 1) * 512],
                                start=True, stop=True)
                            bc_pss.append(bc_ps)
                        bcast_tiles[(it, hp)] = bc_pss
                    else:
                        stile = wp.tile([128, 8], F16, tag="stile")
                        nc.sync.dma_start(out=stile, in_=avr[64:65, 0:1024])
                        stile_r = wp.tile([128, 8], F16, tag="stile_r")
                        with nc.allow_low_precision(reason="f16 softmax denominators"):
                            nc.vector.reciprocal(stile_r, stile)
                        base = (it * 4 + hp) * 1024
                        nc.sync.dma_start(
                            out=bass.AP(tensor=sscratch2, offset=base, ap=[[1, 1024]]),
                            in_=stile_r)
                        bcast = bcp.tile([64, 1024], F16, tag="bcast")
                        nc.sync.dma_start(
                            out=bcast,
                            in_=bass.AP(tensor=sscratch2, offset=base,
                                        ap=[[0, 64], [1, 1024]]))
                        bcast_tiles[(it, hp)] = bcast

                    if it < 3:
                        q_proj_tile(hp, it + 1)
                    if it == 0 and hp < 3:
                        k_proj(hp + 1)
                        q_proj_tile(hp + 1, 0)

            norm_mul(3, 2)
            oproj_step(3, 2)
            norm_mul(3, 3)
            oproj_step(3, 3)

    _split_excess_waits(nc)
    return nc


_CACHED = None


def kernel(x, context, Wq, Wk, Wv, Wo, bo):
    global _CACHED
    if _CACHED is None:
        _CACHED = _build()
    nc = _CACHED

    x = np.asarray(x, dtype=np.float32)
    context = np.asarray(context, dtype=np.float32)
    xf = x.reshape(B, EQ, 64 * 64)
    cf = context.reshape(B, EK, 32 * 32)
    WqT = np.asarray(Wq, np.float32).T.astype(np.float16)   # [EQ, 512]
    WkT = np.asarray(Wk, np.float32).T.astype(np.float16)   # [EK, 512]
    WvT = np.asarray(Wv, np.float32).T.astype(np.float16)   # [EK, 512]
    WoT = np.asarray(Wo, np.float32).T.astype(np.float16)   # [512, OC]
    bo = np.asarray(bo, np.float32)

    # blob1 per batch: ctx chunks + WkT chunks
    def chunks(a, n):
        # [n*128, F] -> [128, n*F] with chunk-major free layout
        return a.reshape(n, 128, -1).transpose(1, 0, 2).reshape(128, -1)

    wk4 = WkT.reshape(4, 128, 4, 128)     # [ec, p, hp, 128]
    wq2 = WqT.reshape(2, 128, 4, 128)     # [ec, p, hp, 128]
    wkt_hp0 = wk4[:, :, 0].transpose(1, 0, 2).reshape(128, 512)
    wqt_hp0 = wq2[:, :, 0].transpose(1, 0, 2).reshape(128, 256)
    wkt_r = wk4[:, :, 1:].transpose(1, 0, 2, 3).reshape(128, 1536)
    wqt_r = wq2[:, :, 1:].transpose(1, 0, 2, 3).reshape(128, 768)
    blobB1 = np.ascontiguousarray(
        np.concatenate([chunks(WvT, 4), wkt_r, wqt_r], axis=1))
    bo2 = np.ascontiguousarray(bo.reshape(2, 128).T)

    in_maps = []
    for core in range(8):
        b, half = core // 2, core % 2
        xh = xf[b, :, half * NQ:(half + 1) * NQ].astype(np.float16).reshape(2, 128, 4, 512)
        x_it0 = xh[:, :, 0].transpose(1, 0, 2).reshape(128, 1024)
        x_r = xh[:, :, 1:].transpose(1, 0, 2, 3).reshape(128, 3072)
        blobA = np.ascontiguousarray(np.concatenate(
            [chunks(cf[b].astype(np.float16), 4), wkt_hp0], axis=1))
        blobA2 = np.ascontiguousarray(np.concatenate([wqt_hp0, x_it0], axis=1))
        blobB2 = np.ascontiguousarray(
            np.concatenate([x_r, chunks(WoT, 4)], axis=1))
        in_maps.append({
            "blobA": blobA, "blobA2": blobA2, "blobB1": blobB1, "blobB2": blobB2,
            "bo2": bo2,
        })

    res = run_bass_kernel_spmd(nc, in_maps, list(range(8)))
    kernel.last_results = res

    out = np.empty((B, OC, 64 * 64), dtype=np.float32)
    for core in range(8):
        b, half = core // 2, core % 2
        out[b, :, half * NQ:(half + 1) * NQ] = res.results[core]["y"]
    return out.reshape(B, OC, 64, 64)
